# revision 4
# baseline (speedup 1.0000x reference)
"""Trainium2 Bass kernel for nn_Attention_23433341567267 (sparse_attention).

5 masked-softmax score pipelines over (B=8, H=12, S=512, D=64) plus one
attention-output matmul.  Sharded: core b handles batch b (all 12 heads).

Device-side math per (head, pipeline), all computed k-major (transposed):
  sT[k,q]  = B[k,:] . A[q,:]        (PE, f32r, d=64 contraction)
           + maskbias^T[k,q]        (PE inject: identity @ maskT, bf16)
  ET[k,q]  = exp(sT)                (ACT -> bf16; masked entries exp(-1e9)=0)
  sums[q]  = sum_k ET[k,q]          (PE: ones-column matmul; for pipeline 0
                                     fused into the PV matmul as an extra
                                     ones column of V)
  outT'[d,q] = sum_k V'[k,d] ET[k,q]  (pipeline 0 only; V' = [V | 1])
  rec[q]   = 1/sums[q]              (DVE)
  bcast    = ones[128] x rec        (PE, K=1 matmul)
  PT[k,q]  = ET * bcast             (DVE -> bf16, DMA out)
  outT[d,q]= outT' * bcast          (DVE -> f32, DMA out)

Host side does sharding/layout only: transposes to d-major, 1/sqrt(D)
pre-scale folded into the q-side operands, bf16 conversion of V/mask,
and the final transpose-back + f32 upcast of the gathered outputs.
"""

import numpy as np
import ml_dtypes

B, H, S, D = 8, 12, 512, 64
NCORES = 8
KC = S // 128  # k-chunks per head
# (A_idx, B_idx) into the stacked operand tensor
# [0]=qT*scale [1]=kT [2]=xo1T*scale [3]=xo2T [4]=xp1T*scale [5]=xp2T
PIPES = [(0, 1), (2, 3), (2, 5), (4, 3), (4, 5)]

_CACHE = {}


def _build_nc():
    import concourse.mybir as mybir
    import concourse.tile as tile
    from concourse import bacc
    from concourse.bass import ts

    f32 = mybir.dt.float32
    f32r = mybir.dt.float32r
    bf16 = mybir.dt.bfloat16
    Exp = mybir.ActivationFunctionType.Exp

    nc = bacc.Bacc("TRN2", target_bir_lowering=False, debug=False,
                   num_devices=NCORES)
    opsT = nc.declare_dram_parameter("opsT", [6, H, D, S], f32r, isOutput=False)
    vext = nc.declare_dram_parameter("vext", [H, S, D + 1], bf16, isOutput=False)
    maskT = nc.declare_dram_parameter("maskT", [S, S], bf16, isOutput=False)
    consts = nc.declare_dram_parameter("consts", [128, 256], bf16, isOutput=False)
    pT = nc.declare_dram_parameter("pT", [5, H, S, S], bf16, isOutput=True)
    outT = nc.declare_dram_parameter("outT", [H, D, S], f32, isOutput=True)

    with tile.TileContext(nc) as tc:
        with (
            tc.tile_pool(name="const", bufs=1) as const_pool,
            tc.tile_pool(name="ops", bufs=2) as ops_pool,
            tc.tile_pool(name="v", bufs=2) as v_pool,
            tc.tile_pool(name="et", bufs=6) as et_pool,
            tc.tile_pool(name="pt", bufs=8) as pt_pool,
            tc.tile_pool(name="rec", bufs=4) as rec_pool,
            tc.tile_pool(name="oc", bufs=2) as oc_pool,
            tc.tile_pool(name="on", bufs=2) as on_pool,
            tc.tile_pool(name="ps", bufs=3, space="PSUM") as ps_pool,
            tc.tile_pool(name="sum", bufs=2, space="PSUM") as sum_pool,
            tc.tile_pool(name="pb", bufs=2, space="PSUM") as pb_pool,
        ):
            ct = const_pool.tile([128, 256], bf16)
            nc.sync.dma_start(ct[:], consts[:])
            mt = const_pool.tile([128, KC, S], bf16)
            nc.sync.dma_start(mt[:], maskT.rearrange("(c p) q -> p c q", p=128))
            ident = ct[:, 0:128]
            ones_col = ct[:, 128:129]
            ones_row = ct[0:1, 128:256]

            for hp in range(H // 2):
                ops6 = ops_pool.tile([128, 6, S], f32r, tag="ops")
                nc.sync.dma_start(
                    ops6[:],
                    opsT[:, 2 * hp : 2 * hp + 2].rearrange("t a p f -> (a p) t f"),
                )
                v8 = v_pool.tile([128, 2, KC, D + 1], bf16, tag="v")
                nc.sync.dma_start(
                    v8[:],
                    vext[2 * hp : 2 * hp + 2].rearrange("h (c p) d -> p h c d", p=128),
                )
                for hh in range(2):
                    h = 2 * hp + hh
                    lo, hi = 64 * hh, 64 * hh + 64
                    for p, (ia, ib) in enumerate(PIPES):
                        et4 = et_pool.tile([128, KC, S], bf16, tag="et")
                        for kc in range(KC):
                            ps = ps_pool.tile([128, S], f32, tag="ps")
                            nc.tensor.matmul(
                                ps[:], lhsT=ident, rhs=mt[:, kc, :],
                                start=True, stop=False,
                            )
                            nc.tensor.matmul(
                                ps[:],
                                lhsT=ops6[lo:hi, ib, ts(kc, 128)],
                                rhs=ops6[lo:hi, ia, :],
                                start=False, stop=True,
                            )
                            nc.scalar.activation(et4[:, kc, :], ps[:], Exp)
                        if p == 0:
                            po = sum_pool.tile([D + 1, S], f32, tag="sum")
                            for kc in range(KC):
                                nc.tensor.matmul(
                                    po[:], lhsT=v8[:, hh, kc, :], rhs=et4[:, kc, :],
                                    start=(kc == 0), stop=(kc == KC - 1),
                                )
                            sums = po[D : D + 1, :]
                        else:
                            po = sum_pool.tile([1, S], f32, tag="sum")
                            for kc in range(KC):
                                nc.tensor.matmul(
                                    po[:], lhsT=ones_col, rhs=et4[:, kc, :],
                                    start=(kc == 0), stop=(kc == KC - 1),
                                )
                            sums = po[0:1, :]
                        rec = rec_pool.tile([1, S], bf16, tag="rec")
                        with nc.allow_low_precision(reason="bf16 softmax recip"):
                            nc.vector.reciprocal(rec[:], sums)
                        pb = pb_pool.tile([128, S], f32, tag="pb")
                        nc.tensor.matmul(
                            pb[:], lhsT=ones_row, rhs=rec[:],
                            start=True, stop=True,
                        )
                        pt4 = pt_pool.tile([128, KC, S], bf16, tag="pt")
                        for kc in range(KC):
                            nc.vector.tensor_mul(pt4[:, kc, :], et4[:, kc, :], pb[:])
                        nc.sync.dma_start(
                            pT[p, h].rearrange("(c p) q -> p c q", p=128), pt4[:]
                        )
                        if p == 0:
                            oc = oc_pool.tile([D, S], f32, tag="oc")
                            nc.scalar.copy(oc[:], po[0:D, :])
                            on = on_pool.tile([D, S], f32, tag="on")
                            nc.vector.tensor_mul(on[:], oc[:], pb[0:D, :])
                            nc.sync.dma_start(outT[h], on[:])
    nc.finalize()
    return nc


def _get_nc():
    if "nc" not in _CACHE:
        _CACHE["nc"] = _build_nc()
    return _CACHE["nc"]


def _prep_core_inputs(b, query, key, value, mask, x_original1, x_original2,
                      x_position1, x_position2):
    bf = ml_dtypes.bfloat16
    scale = np.float32(1.0 / np.sqrt(D))

    def t_(x, s=False):
        x = x[b].astype(np.float32)
        if s:
            x = x * scale
        return np.ascontiguousarray(x.transpose(0, 2, 1))  # [H, D, S]

    opsT = np.ascontiguousarray(np.stack([
        t_(query, True), t_(key),
        t_(x_original1, True), t_(x_original2),
        t_(x_position1, True), t_(x_position2),
    ]))  # [6, H, D, S] f32
    vext = np.ascontiguousarray(np.concatenate(
        [value[b].astype(np.float32), np.ones((H, S, 1), np.float32)], axis=-1
    ).astype(bf))  # [H, S, D+1]
    mb = (mask[b, 0].T.astype(np.float32) - 1.0) * np.float32(1e9)
    maskT = np.ascontiguousarray(mb.astype(bf))  # [S, S]
    consts = np.ascontiguousarray(np.concatenate(
        [np.eye(128, dtype=np.float32), np.ones((128, 128), np.float32)], axis=1
    ).astype(bf))
    return dict(opsT=opsT, vext=vext, maskT=maskT, consts=consts)


def kernel(query, key, value, mask, x_original1, x_original2, x_position1,
           x_position2, _run_kwargs=None):
    from concourse.bass_utils import run_bass_kernel_spmd

    nc = _get_nc()
    in_maps = [
        _prep_core_inputs(b, query, key, value, mask, x_original1, x_original2,
                          x_position1, x_position2)
        for b in range(B)
    ]
    kw = _run_kwargs or {}
    res = run_bass_kernel_spmd(nc, in_maps, list(range(NCORES)), **kw)
    _CACHE["last_result"] = res

    out = np.empty((B, H, S, D), np.float32)
    ps = [np.empty((B, H, S, S), np.float32) for _ in range(5)]
    for b in range(B):
        r = res.results[b]
        out[b] = np.asarray(r["outT"]).transpose(0, 2, 1)
        pTb = np.asarray(r["pT"]).astype(np.float32)  # [5, H, S, S] (k, q)
        for j in range(5):
            ps[j][b] = pTb[j].transpose(0, 2, 1)
    return (out, ps[0], ps[1], ps[2], ps[3], ps[4])
